# revision 30
# baseline (speedup 1.0000x reference)
"""Masked multi-head attention (B=8, N=1024, C=512, H=8) on 8 TRN2 NeuronCores.

Strategy: pure data parallel — core b computes batch element b. All compute is
done in "feature-major" (transposed) layout so no on-chip transposes are needed:

  xT [C, N] --(qkv_w.T tiles as lhsT)--> qkT [1024, N]  (q,k features x tokens)
  v computed token-major [N, 512] directly (lhsT = xT tiles)
  biases folded into the projections as K=1 rank-1 matmul accumulates
  scores^T[k, q] = kT_tile.T @ qT  (contraction over head dim, 64 partitions)
  p^T = exp(scale * s^T)  (ACT; no max subtraction: |scale*s| < 3 here)
  p^T *= keep^T  (bernoulli keep mask, bf16, DVE + GpSimd)
  out^T[d, q] (+ rowsum in row 64) = sum_k v_aug.T @ p^T  (ones column trick)
  rowsums broadcast per head into S [C, N]; S <- 1/S and aT <- aT * S per
  c-tile as head pairs complete; yT = proj (lhsT = proj_w.T tiles).
  Host transposes y^T back.

Everything is software-pipelined at kt granularity (scores/exp/mask of head h
interleaved with PV of head h-1; keep-mask DMA prefetched 2 heads ahead) and
each kt gets its own SBUF/PSUM tile so the Tile scheduler's tile-granular
dependency tracking cannot create false serialization chains.

The bernoulli mask is reproduced on host with jax.random (same PRNG impl the
reference uses) and streamed as a bf16 0/1 keep tensor — the dominant memory
traffic (memory-bound problem).
"""

import numpy as np

B, N, C, H = 8, 1024, 512, 8
HD = C // H  # 64
SCALE = HD**-0.5
NT = N // 128  # 8 token tiles
CT = C // 128  # 4 feature tiles
GP_KT = ()  # gpsimd does broadcasts only: mixing op types on it stalls the pipeline

_cache = {}


def _bf16():
    import ml_dtypes

    return ml_dtypes.bfloat16


def _build_nc():
    from concourse import bacc, mybir, tile

    f32 = mybir.dt.float32
    bf16 = mybir.dt.bfloat16
    MULT = mybir.AluOpType.mult
    ID = mybir.ActivationFunctionType.Identity

    nc = bacc.Bacc("TRN2", target_bir_lowering=False, debug=False, num_devices=8)

    xT_d = nc.declare_dram_parameter("xT", [C, N], bf16, isOutput=False)
    qkvw_d = nc.declare_dram_parameter("qkvw", [C, 3 * C], bf16, isOutput=False)
    projw_d = nc.declare_dram_parameter("projw", [C, C], bf16, isOutput=False)
    keep_d = nc.declare_dram_parameter("keepT", [H, N, N], bf16, isOutput=False)
    qkbr_d = nc.declare_dram_parameter("qkbr", [1, 2 * C], bf16, isOutput=False)
    bvr_d = nc.declare_dram_parameter("bvr", [1, C], bf16, isOutput=False)
    pjb_d = nc.declare_dram_parameter("pjb", [128, 4], f32, isOutput=False)
    out_d = nc.declare_dram_parameter("out", [C, N], f32, isOutput=True)

    with tile.TileContext(nc) as tc:
        with (
            tc.tile_pool(name="const", bufs=1) as cpool,
            tc.tile_pool(name="qk", bufs=1) as qkpool,
            tc.tile_pool(name="vp", bufs=1) as vpool,
            tc.tile_pool(name="ap", bufs=1) as apool,
            tc.tile_pool(name="sp", bufs=1) as spool,
            tc.tile_pool(name="yp", bufs=1) as ypool,
            tc.tile_pool(name="keep", bufs=12) as kpool,
            tc.tile_pool(name="pp", bufs=12) as ppool,
            tc.tile_pool(name="rs", bufs=2) as rspool,
            tc.tile_pool(name="scps", bufs=2, space="PSUM") as scpool,
            tc.tile_pool(name="acc", bufs=2, space="PSUM") as accpool,
        ):
            # ---- inputs / constants ----
            # xt split into column halves, qkvw into 4 m-column groups + v, so
            # the first matmuls only wait on the slices they need.
            xT_ap = xT_d[:].rearrange("(t p) n -> p t n", p=128)
            xts = []
            for n2 in range(2):
                xth = cpool.tile([128, CT, 512], bf16, tag=f"xt{n2}", name=f"xt{n2}")
                nc.sync.dma_start(xth[:], xT_ap[:, :, n2 * 512 : (n2 + 1) * 512])
                xts.append(xth)
            qkvw_ap = qkvw_d[:].rearrange("(t p) n -> p t n", p=128)
            qkws = []
            for g in range(4):
                w = cpool.tile([128, CT, 256], bf16, tag=f"qkw{g}", name=f"qkw{g}")
                nc.sync.dma_start(w[:], qkvw_ap[:, :, g * 256 : (g + 1) * 256])
                qkws.append(w)
            qkbr = cpool.tile([1, 2 * C], bf16)
            nc.sync.dma_start(qkbr[:], qkbr_d[:])
            bvr = cpool.tile([1, C], bf16)
            nc.sync.dma_start(bvr[:], bvr_d[:])
            pjb = cpool.tile([128, 4], f32)
            nc.sync.dma_start(pjb[:], pjb_d[:])
            ones = cpool.tile([1, 512], bf16)
            nc.gpsimd.memset(ones[:], 1.0)
            vw = cpool.tile([128, CT, C], bf16, tag="vw", name="vw")
            nc.sync.dma_start(vw[:], qkvw_ap[:, :, 2 * C : 3 * C])

            keep_ap = keep_d[:].rearrange("h (kp p) q -> p h kp q", p=128)
            keeps = {}

            def prefetch_keep(h):
                for q4 in range(4):
                    keep2 = kpool.tile(
                        [128, 2, N], bf16, tag="keep", name=f"keep{h}_{q4}"
                    )
                    nc.sync.dma_start(keep2[:], keep_ap[:, h, q4 * 2 : q4 * 2 + 2, :])
                    keeps[(h, q4)] = keep2

            prefetch_keep(1)
            prefetch_keep(0)

            # ---- QKV projections (bias via K=1 rank-1 accumulate) ----
            # one SBUF tile per m so scores of early heads never falsely
            # serialize against later m-tile evacuations (tile-granular deps)
            qkTs = [None] * 8

            def emit_qk_tile(m):
                ps = accpool.tile([128, N], f32, tag="acc", name=f"qkps{m}")
                for t in range(CT):
                    lhsT = qkws[m // 2][:, t, (m % 2) * 128 : (m % 2) * 128 + 128]
                    for n2 in range(2):
                        nc.tensor.matmul(
                            ps[:, n2 * 512 : (n2 + 1) * 512],
                            lhsT,
                            xts[n2][:, t, :],
                            start=(t == 0),
                            stop=False,
                        )
                for n2 in range(2):
                    nc.tensor.matmul(
                        ps[:, n2 * 512 : (n2 + 1) * 512],
                        qkbr[0:1, m * 128 : (m + 1) * 128],
                        ones[0:1, :],
                        start=False,
                        stop=True,
                    )
                qkTs[m] = qkpool.tile([128, N], bf16, tag=f"qk{m}", name=f"qkT{m}")
                nc.scalar.activation(qkTs[m][:], ps[:], ID, bias=0.0, scale=1.0)

            emit_qk_tile(0)
            emit_qk_tile(4)

            # v token-major with ones column: v[tok, head, 66]
            # (matmuls emitted inside head-0's loop as PE filler work)
            vsbs = [None] * NT

            def emit_v_tile(mt):
                psv = accpool.tile([128, N], f32, tag="acc", name=f"vps{mt}")
                for t in range(CT):
                    nc.tensor.matmul(
                        psv[:, 0:512],
                        xts[mt // 4][:, t, (mt % 4) * 128 : (mt % 4) * 128 + 128],
                        vw[:, t, :],
                        start=(t == 0),
                        stop=False,
                    )
                nc.tensor.matmul(
                    psv[:, 0:512], ones[0:1, 0:128], bvr[0:1, :], start=False, stop=True
                )
                vsb = vpool.tile([128, H, 66], bf16, tag=f"v{mt}", name=f"vsb{mt}")
                vsbs[mt] = vsb
                nc.gpsimd.memset(vsb[:, :, 64:65], 1.0)
                nc.gpsimd.memset(vsb[:, :, 65:66], 0.0)
                if mt % 2 == 0:
                    nc.scalar.activation(
                        vsb[:, :, 0:64],
                        psv[:, 0:512].rearrange("p (h d) -> p h d", d=64),
                        ID,
                        bias=0.0,
                        scale=1.0,
                    )
                else:
                    nc.vector.tensor_copy(
                        vsb[:, :, 0:64],
                        psv[:, 0:512].rearrange("p (h d) -> p h d", d=64),
                    )

            # ---- attention: software-pipelined heads, kt granularity ----
            aTs = [
                apool.tile([128, N], bf16, tag=f"aT{t}", name=f"aT{t}")
                for t in range(CT)
            ]
            Ss = [
                spool.tile([128, N], f32, tag=f"S{t}", name=f"S{t}") for t in range(CT)
            ]
            pts = {}  # (h, kt) -> [128, N] bf16 masked-p tile
            psos = [None] * H

            def emit_score_kt(h, kt):
                pbase = (h % 2) * 64
                qT_h = qkTs[h // 2][pbase : pbase + 64, :]
                kT_h = qkTs[4 + h // 2][pbase : pbase + 64, :]
                ps_s = scpool.tile([128, N], f32, tag="sc", name=f"sc{h}_{kt}")
                for n2 in range(2):
                    nc.tensor.matmul(
                        ps_s[:, n2 * 512 : (n2 + 1) * 512],
                        kT_h[:, kt * 128 : (kt + 1) * 128],
                        qT_h[:, n2 * 512 : (n2 + 1) * 512],
                        start=True,
                        stop=True,
                    )
                pt = ppool.tile([128, N], bf16, tag="pT", name=f"pT{h}_{kt}")
                pts[(h, kt)] = pt
                nc.scalar.activation(
                    pt[:],
                    ps_s[:],
                    mybir.ActivationFunctionType.Exp,
                    bias=0.0,
                    scale=float(SCALE),
                )
                eng = nc.gpsimd if kt in GP_KT else nc.vector
                eng.tensor_tensor(pt[:], pt[:], keeps[(h, kt // 2)][:, kt % 2, :], MULT)

            def emit_pv_kt(h, kt):
                if kt == 0:
                    psos[h] = accpool.tile([128, N], f32, tag="acc", name=f"pso{h}")
                ps_o = psos[h]
                for n2 in range(2):
                    nc.tensor.matmul(
                        ps_o[0:66, n2 * 512 : (n2 + 1) * 512],
                        vsbs[kt][:, h, :],
                        pts[(h, kt)][:, n2 * 512 : (n2 + 1) * 512],
                        start=(kt == 0),
                        stop=(kt == NT - 1 and n2 == 1),
                    )
                if kt == NT - 1:
                    del pts[(h, kt)]

            def emit_head_tail(h, norm_pair):
                """rowsum -> partition 0, broadcast into S block, evac out^T."""
                ps_o = psos[h]
                pbase = (h % 2) * 64
                rs = rspool.tile([1, N], f32, tag="rs")
                nc.vector.tensor_copy(rs[0:1, :], ps_o[64:65, :])
                if pbase == 0:
                    nc.gpsimd.partition_broadcast(Ss[h // 2][0:64, :], rs[0:1, :])
                else:
                    rb = rspool.tile([64, N], f32, tag="rb")
                    nc.gpsimd.partition_broadcast(rb[:], rs[0:1, :])
                    nc.sync.dma_start(Ss[h // 2][64:128, :], rb[:])
                nc.vector.tensor_copy(aTs[h // 2][pbase : pbase + 64, :], ps_o[0:64, :])
                psos[h] = None
                if norm_pair:
                    # heads 2t and 2t+1 have filled c-tile t: normalize it now
                    t = h // 2
                    nc.vector.reciprocal_approx_fast(Ss[t][:], Ss[t][:])
                    nc.vector.tensor_tensor(aTs[t][:], aTs[t][:], Ss[t][:], MULT)

            # head order: pair-swapped so the final head is even (cheaper
            # tail chain) and the last head's PV self-interleaves at lag 3
            ORDER = [1, 0, 3, 2, 5, 4, 7, 6]
            QK_FOLD = {0: 1, 1: 5, 2: 2, 3: 6, 4: 3, 5: 7}
            for i, h in enumerate(ORDER):
                if i + 2 < H:
                    prefetch_keep(ORDER[i + 2])
                last = i == H - 1
                for kt in range(NT):
                    emit_score_kt(h, kt)
                    if i == 0:
                        emit_v_tile(kt)
                    else:
                        emit_pv_kt(ORDER[i - 1], kt)
                    if last and kt >= 2:
                        emit_pv_kt(h, kt - 2)
                    if kt == 6 and i in QK_FOLD:
                        emit_qk_tile(QK_FOLD[i])
                if i > 0:
                    emit_head_tail(ORDER[i - 1], i % 2 == 0)
            for kt in range(NT - 2, NT):
                emit_pv_kt(ORDER[-1], kt)
            emit_head_tail(ORDER[-1], True)

            # ---- output projection ----
            projw = cpool.tile([128, CT, C], bf16, tag="projw", name="projw")
            nc.sync.dma_start(
                projw[:], projw_d[:].rearrange("(t p) n -> p t n", p=128)
            )
            # t-outer accumulation into 4 parallel PSUM tiles so the t<3
            # matmuls overlap the final head's tail/normalization.
            ps_ys = []
            for mo in range(CT):
                pool = scpool if mo < 2 else accpool
                tag = "sc" if mo < 2 else "acc"
                ps_ys.append(pool.tile([128, N], f32, tag=tag, name=f"yps{mo}"))
            for t in range(CT):
                for mo in range(CT):
                    lhsT = projw[:, t, mo * 128 : (mo + 1) * 128]
                    for n2 in range(2):
                        nc.tensor.matmul(
                            ps_ys[mo][:, n2 * 512 : (n2 + 1) * 512],
                            lhsT,
                            aTs[t][:, n2 * 512 : (n2 + 1) * 512],
                            start=(t == 0),
                            stop=(t == CT - 1),
                        )
            for mo in range(CT):
                yT = ypool.tile([128, N], f32, tag="yT", name=f"yT{mo}", bufs=2)
                nc.scalar.activation(
                    yT[:], ps_ys[mo][:], ID, bias=pjb[:, mo : mo + 1], scale=1.0
                )
                nc.sync.dma_start(
                    out_d[:].rearrange("(t p) n -> p t n", p=128)[:, mo, :], yT[:]
                )

    nc.compile()
    return nc


def _get_nc():
    if "nc" not in _cache:
        _cache["nc"] = _build_nc()
    return _cache["nc"]


def _get_keepT():
    """keepT[b, h, k, q] = 1 - mask[b, h, q, k], bf16, one array per batch."""
    if "keepT" not in _cache:
        import jax

        mask = np.asarray(jax.random.bernoulli(jax.random.key(42), 0.5, (B, H, N, N)))
        keep = (~mask).astype(_bf16())
        _cache["keepT"] = np.ascontiguousarray(keep.transpose(0, 1, 3, 2))
    return _cache["keepT"]


def kernel(x, qkv_w, qkv_b, proj_w, proj_b):
    from concourse.bass_utils import run_bass_kernel_spmd

    bf16 = _bf16()
    x = np.asarray(x, dtype=np.float32)
    qkv_w = np.asarray(qkv_w, dtype=np.float32)
    qkv_b = np.asarray(qkv_b, dtype=np.float32)
    proj_w = np.asarray(proj_w, dtype=np.float32)
    proj_b = np.asarray(proj_b, dtype=np.float32)

    nc = _get_nc()
    keepT = _get_keepT()

    qkvw_t = np.ascontiguousarray(qkv_w.T).astype(bf16)  # [C, 3C]
    projw_t = np.ascontiguousarray(proj_w.T).astype(bf16)  # [C, C]
    qkbr = qkv_b[:1024].astype(bf16).reshape(1, 2 * C)
    bvr = qkv_b[1024:].astype(bf16).reshape(1, C)
    pjb = np.ascontiguousarray(proj_b.reshape(4, 128).T)  # [128, 4]

    in_maps = []
    for b in range(B):
        in_maps.append(
            {
                "xT": np.ascontiguousarray(x[b].T).astype(bf16),
                "qkvw": qkvw_t,
                "projw": projw_t,
                "keepT": keepT[b],
                "qkbr": qkbr,
                "bvr": bvr,
                "pjb": pjb,
            }
        )

    global _last_in_maps
    _last_in_maps = in_maps
    res = run_bass_kernel_spmd(nc, in_maps, core_ids=list(range(8)))
    out = np.stack([res.results[b]["out"].T for b in range(B)])  # [B, N, C]
    return out.astype(np.float32)


_last_in_maps = None


# revision 31
# speedup vs baseline: 1.0152x; 1.0152x over previous
"""Masked multi-head attention (B=8, N=1024, C=512, H=8) on 8 TRN2 NeuronCores.

Strategy: pure data parallel — core b computes batch element b. All compute is
done in "feature-major" (transposed) layout so no on-chip transposes are needed:

  xT [C, N] --(qkv_w.T tiles as lhsT)--> qkT [1024, N]  (q,k features x tokens)
  v computed token-major [N, 512] directly (lhsT = xT tiles)
  biases folded into the projections as K=1 rank-1 matmul accumulates
  scores^T[k, q] = kT_tile.T @ qT  (contraction over head dim, 64 partitions)
  p^T = exp(scale * s^T)  (ACT; no max subtraction: |scale*s| < 3 here)
  p^T *= keep^T  (bernoulli keep mask, bf16, DVE + GpSimd)
  out^T[d, q] (+ rowsum in row 64) = sum_k v_aug.T @ p^T  (ones column trick)
  rowsums broadcast per head into S [C, N]; S <- 1/S and aT <- aT * S per
  c-tile as head pairs complete; yT = proj (lhsT = proj_w.T tiles).
  Host transposes y^T back.

Everything is software-pipelined at kt granularity (scores/exp/mask of head h
interleaved with PV of head h-1; keep-mask DMA prefetched 2 heads ahead) and
each kt gets its own SBUF/PSUM tile so the Tile scheduler's tile-granular
dependency tracking cannot create false serialization chains.

The bernoulli mask is reproduced on host with jax.random (same PRNG impl the
reference uses) and streamed as a bf16 0/1 keep tensor — the dominant memory
traffic (memory-bound problem).
"""

import numpy as np

B, N, C, H = 8, 1024, 512, 8
HD = C // H  # 64
SCALE = HD**-0.5
NT = N // 128  # 8 token tiles
CT = C // 128  # 4 feature tiles
GP_KT = ()  # gpsimd does broadcasts only: mixing op types on it stalls the pipeline

_cache = {}


def _bf16():
    import ml_dtypes

    return ml_dtypes.bfloat16


def _build_nc():
    from concourse import bacc, mybir, tile

    f32 = mybir.dt.float32
    bf16 = mybir.dt.bfloat16
    MULT = mybir.AluOpType.mult
    ID = mybir.ActivationFunctionType.Identity

    nc = bacc.Bacc("TRN2", target_bir_lowering=False, debug=False, num_devices=8)

    xT_d = nc.declare_dram_parameter("xT", [C, N], bf16, isOutput=False)
    qkvw_d = nc.declare_dram_parameter("qkvw", [C, 3 * C], bf16, isOutput=False)
    projw_d = nc.declare_dram_parameter("projw", [C, C], bf16, isOutput=False)
    keep_d = nc.declare_dram_parameter("keepT", [H, N, N], bf16, isOutput=False)
    qkbr_d = nc.declare_dram_parameter("qkbr", [1, 2 * C], bf16, isOutput=False)
    bvr_d = nc.declare_dram_parameter("bvr", [1, C], bf16, isOutput=False)
    pjb_d = nc.declare_dram_parameter("pjb", [128, 4], f32, isOutput=False)
    out_d = nc.declare_dram_parameter("out", [C, N], f32, isOutput=True)

    with tile.TileContext(nc) as tc:
        with (
            tc.tile_pool(name="const", bufs=1) as cpool,
            tc.tile_pool(name="qk", bufs=1) as qkpool,
            tc.tile_pool(name="vp", bufs=1) as vpool,
            tc.tile_pool(name="ap", bufs=1) as apool,
            tc.tile_pool(name="sp", bufs=1) as spool,
            tc.tile_pool(name="yp", bufs=1) as ypool,
            tc.tile_pool(name="keep", bufs=12) as kpool,
            tc.tile_pool(name="pp", bufs=12) as ppool,
            tc.tile_pool(name="rs", bufs=2) as rspool,
            tc.tile_pool(name="scps", bufs=2, space="PSUM") as scpool,
            tc.tile_pool(name="acc", bufs=2, space="PSUM") as accpool,
        ):
            # ---- inputs / constants ----
            # xt split into column halves, qkvw into 4 m-column groups + v, so
            # the first matmuls only wait on the slices they need.
            xT_ap = xT_d[:].rearrange("(t p) n -> p t n", p=128)
            xts = []
            for n2 in range(2):
                xth = cpool.tile([128, CT, 512], bf16, tag=f"xt{n2}", name=f"xt{n2}")
                nc.sync.dma_start(xth[:], xT_ap[:, :, n2 * 512 : (n2 + 1) * 512])
                xts.append(xth)
            qkvw_ap = qkvw_d[:].rearrange("(t p) n -> p t n", p=128)
            qkws = []
            for g in range(4):
                w = cpool.tile([128, CT, 256], bf16, tag=f"qkw{g}", name=f"qkw{g}")
                nc.sync.dma_start(w[:], qkvw_ap[:, :, g * 256 : (g + 1) * 256])
                qkws.append(w)
            qkbr = cpool.tile([1, 2 * C], bf16)
            nc.sync.dma_start(qkbr[:], qkbr_d[:])
            bvr = cpool.tile([1, C], bf16)
            nc.sync.dma_start(bvr[:], bvr_d[:])
            pjb = cpool.tile([128, 4], f32)
            nc.sync.dma_start(pjb[:], pjb_d[:])
            ones = cpool.tile([1, 512], bf16)
            nc.gpsimd.memset(ones[:], 1.0)
            vw = cpool.tile([128, CT, C], bf16, tag="vw", name="vw")
            nc.sync.dma_start(vw[:], qkvw_ap[:, :, 2 * C : 3 * C])

            keep_ap = keep_d[:].rearrange("h (kp p) q -> p h kp q", p=128)
            keeps = {}

            def prefetch_keep(h):
                for q4 in range(4):
                    keep2 = kpool.tile(
                        [128, 2, N], bf16, tag="keep", name=f"keep{h}_{q4}"
                    )
                    nc.sync.dma_start(keep2[:], keep_ap[:, h, q4 * 2 : q4 * 2 + 2, :])
                    keeps[(h, q4)] = keep2

            prefetch_keep(1)
            prefetch_keep(0)

            # ---- QKV projections (bias via K=1 rank-1 accumulate) ----
            # one SBUF tile per m so scores of early heads never falsely
            # serialize against later m-tile evacuations (tile-granular deps)
            qkTs = [None] * 8

            qk_ps = {}

            def emit_qk_half(m, half):
                if half == 0:
                    qk_ps[m] = accpool.tile([128, N], f32, tag="acc", name=f"qkps{m}")
                ps = qk_ps[m]
                for t in (0, 1) if half == 0 else (2, 3):
                    lhsT = qkws[m // 2][:, t, (m % 2) * 128 : (m % 2) * 128 + 128]
                    for n2 in range(2):
                        nc.tensor.matmul(
                            ps[:, n2 * 512 : (n2 + 1) * 512],
                            lhsT,
                            xts[n2][:, t, :],
                            start=(t == 0),
                            stop=False,
                        )
                if half == 1:
                    for n2 in range(2):
                        nc.tensor.matmul(
                            ps[:, n2 * 512 : (n2 + 1) * 512],
                            qkbr[0:1, m * 128 : (m + 1) * 128],
                            ones[0:1, :],
                            start=False,
                            stop=True,
                        )
                    qkTs[m] = qkpool.tile(
                        [128, N], bf16, tag=f"qk{m}", name=f"qkT{m}"
                    )
                    nc.scalar.activation(qkTs[m][:], ps[:], ID, bias=0.0, scale=1.0)

            def emit_qk_tile(m):
                emit_qk_half(m, 0)
                emit_qk_half(m, 1)

            emit_qk_tile(0)
            emit_qk_tile(4)

            # v token-major with ones column: v[tok, head, 66]
            # (matmuls emitted inside head-0's loop as PE filler work)
            vsbs = [None] * NT

            def emit_v_tile(mt):
                psv = accpool.tile([128, N], f32, tag="acc", name=f"vps{mt}")
                for t in range(CT):
                    nc.tensor.matmul(
                        psv[:, 0:512],
                        xts[mt // 4][:, t, (mt % 4) * 128 : (mt % 4) * 128 + 128],
                        vw[:, t, :],
                        start=(t == 0),
                        stop=False,
                    )
                nc.tensor.matmul(
                    psv[:, 0:512], ones[0:1, 0:128], bvr[0:1, :], start=False, stop=True
                )
                vsb = vpool.tile([128, H, 66], bf16, tag=f"v{mt}", name=f"vsb{mt}")
                vsbs[mt] = vsb
                nc.gpsimd.memset(vsb[:, :, 64:65], 1.0)
                nc.gpsimd.memset(vsb[:, :, 65:66], 0.0)
                if mt % 2 == 0:
                    nc.scalar.activation(
                        vsb[:, :, 0:64],
                        psv[:, 0:512].rearrange("p (h d) -> p h d", d=64),
                        ID,
                        bias=0.0,
                        scale=1.0,
                    )
                else:
                    nc.vector.tensor_copy(
                        vsb[:, :, 0:64],
                        psv[:, 0:512].rearrange("p (h d) -> p h d", d=64),
                    )

            # ---- attention: software-pipelined heads, kt granularity ----
            aTs = [
                apool.tile([128, N], bf16, tag=f"aT{t}", name=f"aT{t}")
                for t in range(CT)
            ]
            Ss = [
                spool.tile([128, N], f32, tag=f"S{t}", name=f"S{t}") for t in range(CT)
            ]
            pts = {}  # (h, kt) -> [128, N] bf16 masked-p tile
            psos = [None] * H

            def emit_score_kt(h, kt):
                pbase = (h % 2) * 64
                qT_h = qkTs[h // 2][pbase : pbase + 64, :]
                kT_h = qkTs[4 + h // 2][pbase : pbase + 64, :]
                ps_s = scpool.tile([128, N], f32, tag="sc", name=f"sc{h}_{kt}")
                for n2 in range(2):
                    nc.tensor.matmul(
                        ps_s[:, n2 * 512 : (n2 + 1) * 512],
                        kT_h[:, kt * 128 : (kt + 1) * 128],
                        qT_h[:, n2 * 512 : (n2 + 1) * 512],
                        start=True,
                        stop=True,
                    )
                pt = ppool.tile([128, N], bf16, tag="pT", name=f"pT{h}_{kt}")
                pts[(h, kt)] = pt
                nc.scalar.activation(
                    pt[:],
                    ps_s[:],
                    mybir.ActivationFunctionType.Exp,
                    bias=0.0,
                    scale=float(SCALE),
                )
                eng = nc.gpsimd if kt in GP_KT else nc.vector
                eng.tensor_tensor(pt[:], pt[:], keeps[(h, kt // 2)][:, kt % 2, :], MULT)

            def emit_pv_kt(h, kt):
                if kt == 0:
                    psos[h] = accpool.tile([128, N], f32, tag="acc", name=f"pso{h}")
                ps_o = psos[h]
                for n2 in range(2):
                    nc.tensor.matmul(
                        ps_o[0:66, n2 * 512 : (n2 + 1) * 512],
                        vsbs[kt][:, h, :],
                        pts[(h, kt)][:, n2 * 512 : (n2 + 1) * 512],
                        start=(kt == 0),
                        stop=(kt == NT - 1 and n2 == 1),
                    )
                if kt == NT - 1:
                    del pts[(h, kt)]

            def emit_head_tail(h, norm_pair):
                """rowsum -> partition 0, broadcast into S block, evac out^T."""
                ps_o = psos[h]
                pbase = (h % 2) * 64
                rs = rspool.tile([1, N], f32, tag="rs")
                nc.vector.tensor_copy(rs[0:1, :], ps_o[64:65, :])
                if pbase == 0:
                    nc.gpsimd.partition_broadcast(Ss[h // 2][0:64, :], rs[0:1, :])
                else:
                    rb = rspool.tile([64, N], f32, tag="rb")
                    nc.gpsimd.partition_broadcast(rb[:], rs[0:1, :])
                    nc.sync.dma_start(Ss[h // 2][64:128, :], rb[:])
                nc.vector.tensor_copy(aTs[h // 2][pbase : pbase + 64, :], ps_o[0:64, :])
                psos[h] = None
                if norm_pair:
                    # heads 2t and 2t+1 have filled c-tile t: normalize it now
                    t = h // 2
                    nc.vector.reciprocal_approx_fast(Ss[t][:], Ss[t][:])
                    nc.vector.tensor_tensor(aTs[t][:], aTs[t][:], Ss[t][:], MULT)

            # head order: pair-swapped so the final head is even (cheaper
            # tail chain) and the last head's PV self-interleaves at lag 3
            ORDER = [1, 0, 3, 2, 5, 4, 7, 6]
            QK_FOLD = {0: 1, 1: 5, 2: 2, 3: 6, 4: 3, 5: 7}
            for i, h in enumerate(ORDER):
                if i + 2 < H:
                    prefetch_keep(ORDER[i + 2])
                last = i == H - 1
                for kt in range(NT):
                    emit_score_kt(h, kt)
                    if i == 0:
                        emit_v_tile(kt)
                    else:
                        emit_pv_kt(ORDER[i - 1], kt)
                    if last and kt >= 2:
                        emit_pv_kt(h, kt - 2)
                    if kt == 3 and i in QK_FOLD:
                        emit_qk_half(QK_FOLD[i], 0)
                    if kt == 6 and i in QK_FOLD:
                        emit_qk_half(QK_FOLD[i], 1)
                if i > 0:
                    emit_head_tail(ORDER[i - 1], i % 2 == 0)
            for kt in range(NT - 2, NT):
                emit_pv_kt(ORDER[-1], kt)
            emit_head_tail(ORDER[-1], True)

            # ---- output projection ----
            projw = cpool.tile([128, CT, C], bf16, tag="projw", name="projw")
            nc.sync.dma_start(
                projw[:], projw_d[:].rearrange("(t p) n -> p t n", p=128)
            )
            # t-outer accumulation into 4 parallel PSUM tiles so the t<3
            # matmuls overlap the final head's tail/normalization.
            ps_ys = []
            for mo in range(CT):
                pool = scpool if mo < 2 else accpool
                tag = "sc" if mo < 2 else "acc"
                ps_ys.append(pool.tile([128, N], f32, tag=tag, name=f"yps{mo}"))
            for t in range(CT):
                for mo in range(CT):
                    lhsT = projw[:, t, mo * 128 : (mo + 1) * 128]
                    for n2 in range(2):
                        nc.tensor.matmul(
                            ps_ys[mo][:, n2 * 512 : (n2 + 1) * 512],
                            lhsT,
                            aTs[t][:, n2 * 512 : (n2 + 1) * 512],
                            start=(t == 0),
                            stop=(t == CT - 1),
                        )
            for mo in range(CT):
                yT = ypool.tile([128, N], f32, tag="yT", name=f"yT{mo}", bufs=4)
                if mo % 2 == 0:
                    nc.scalar.activation(
                        yT[:], ps_ys[mo][:], ID, bias=pjb[:, mo : mo + 1], scale=1.0
                    )
                else:
                    nc.vector.tensor_scalar_add(
                        yT[:], ps_ys[mo][:], pjb[:, mo : mo + 1]
                    )
                nc.sync.dma_start(
                    out_d[:].rearrange("(t p) n -> p t n", p=128)[:, mo, :], yT[:]
                )

    nc.compile()
    return nc


def _get_nc():
    if "nc" not in _cache:
        _cache["nc"] = _build_nc()
    return _cache["nc"]


def _get_keepT():
    """keepT[b, h, k, q] = 1 - mask[b, h, q, k], bf16, one array per batch."""
    if "keepT" not in _cache:
        import jax

        mask = np.asarray(jax.random.bernoulli(jax.random.key(42), 0.5, (B, H, N, N)))
        keep = (~mask).astype(_bf16())
        _cache["keepT"] = np.ascontiguousarray(keep.transpose(0, 1, 3, 2))
    return _cache["keepT"]


def kernel(x, qkv_w, qkv_b, proj_w, proj_b):
    from concourse.bass_utils import run_bass_kernel_spmd

    bf16 = _bf16()
    x = np.asarray(x, dtype=np.float32)
    qkv_w = np.asarray(qkv_w, dtype=np.float32)
    qkv_b = np.asarray(qkv_b, dtype=np.float32)
    proj_w = np.asarray(proj_w, dtype=np.float32)
    proj_b = np.asarray(proj_b, dtype=np.float32)

    nc = _get_nc()
    keepT = _get_keepT()

    qkvw_t = np.ascontiguousarray(qkv_w.T).astype(bf16)  # [C, 3C]
    projw_t = np.ascontiguousarray(proj_w.T).astype(bf16)  # [C, C]
    qkbr = qkv_b[:1024].astype(bf16).reshape(1, 2 * C)
    bvr = qkv_b[1024:].astype(bf16).reshape(1, C)
    pjb = np.ascontiguousarray(proj_b.reshape(4, 128).T)  # [128, 4]

    in_maps = []
    for b in range(B):
        in_maps.append(
            {
                "xT": np.ascontiguousarray(x[b].T).astype(bf16),
                "qkvw": qkvw_t,
                "projw": projw_t,
                "keepT": keepT[b],
                "qkbr": qkbr,
                "bvr": bvr,
                "pjb": pjb,
            }
        )

    global _last_in_maps
    _last_in_maps = in_maps
    res = run_bass_kernel_spmd(nc, in_maps, core_ids=list(range(8)))
    out = np.stack([res.results[b]["out"].T for b in range(B)])  # [B, N, C]
    return out.astype(np.float32)


_last_in_maps = None


# revision 32
# speedup vs baseline: 1.0516x; 1.0359x over previous
"""Masked multi-head attention (B=8, N=1024, C=512, H=8) on 8 TRN2 NeuronCores.

Strategy: pure data parallel — core b computes batch element b. All compute is
done in "feature-major" (transposed) layout so no on-chip transposes are needed:

  xT [C, N] --(qkv_w.T tiles as lhsT)--> qkT [1024, N]  (q,k features x tokens)
  v computed token-major [N, 512] directly (lhsT = xT tiles)
  biases folded into the projections as K=1 rank-1 matmul accumulates
  scores^T[k, q] = kT_tile.T @ qT  (contraction over head dim, 64 partitions)
  p^T = exp(scale * s^T)  (ACT; no max subtraction: |scale*s| < 3 here)
  p^T *= keep^T  (bernoulli keep mask, bf16, DVE + GpSimd)
  out^T[d, q] (+ rowsum in row 64) = sum_k v_aug.T @ p^T  (ones column trick)
  rowsums broadcast per head into S [C, N]; S <- 1/S and aT <- aT * S per
  c-tile as head pairs complete; yT = proj (lhsT = proj_w.T tiles).
  Host transposes y^T back.

Everything is software-pipelined at kt granularity (scores/exp/mask of head h
interleaved with PV of head h-1; keep-mask DMA prefetched 2 heads ahead) and
each kt gets its own SBUF/PSUM tile so the Tile scheduler's tile-granular
dependency tracking cannot create false serialization chains.

The bernoulli mask is reproduced on host with jax.random (same PRNG impl the
reference uses) and streamed as a bf16 0/1 keep tensor — the dominant memory
traffic (memory-bound problem).
"""

import numpy as np

B, N, C, H = 8, 1024, 512, 8
HD = C // H  # 64
SCALE = HD**-0.5
NT = N // 128  # 8 token tiles
CT = C // 128  # 4 feature tiles
GP_KT = ()  # gpsimd does broadcasts only: mixing op types on it stalls the pipeline

_cache = {}


def _bf16():
    import ml_dtypes

    return ml_dtypes.bfloat16


def _build_nc():
    from concourse import bacc, mybir, tile

    f32 = mybir.dt.float32
    bf16 = mybir.dt.bfloat16
    MULT = mybir.AluOpType.mult
    ID = mybir.ActivationFunctionType.Identity

    nc = bacc.Bacc("TRN2", target_bir_lowering=False, debug=False, num_devices=8)

    xT_d = nc.declare_dram_parameter("xT", [C, N], bf16, isOutput=False)
    qkvw_d = nc.declare_dram_parameter("qkvw", [C, 3 * C], bf16, isOutput=False)
    projw_d = nc.declare_dram_parameter("projw", [C, C], bf16, isOutput=False)
    keep_d = nc.declare_dram_parameter("keepT", [H, N, N], bf16, isOutput=False)
    qkbr_d = nc.declare_dram_parameter("qkbr", [1, 2 * C], bf16, isOutput=False)
    bvc_d = nc.declare_dram_parameter("bvc", [128, 4], f32, isOutput=False)
    pjb_d = nc.declare_dram_parameter("pjb", [128, 4], f32, isOutput=False)
    out_d = nc.declare_dram_parameter("out", [C, N], f32, isOutput=True)

    with tile.TileContext(nc) as tc:
        with (
            tc.tile_pool(name="const", bufs=1) as cpool,
            tc.tile_pool(name="qk", bufs=1) as qkpool,
            tc.tile_pool(name="vp", bufs=1) as vpool,
            tc.tile_pool(name="ap", bufs=1) as apool,
            tc.tile_pool(name="sp", bufs=1) as spool,
            tc.tile_pool(name="yp", bufs=1) as ypool,
            tc.tile_pool(name="keep", bufs=12) as kpool,
            tc.tile_pool(name="pp", bufs=12) as ppool,
            tc.tile_pool(name="rs", bufs=2) as rspool,
            tc.tile_pool(name="scps", bufs=2, space="PSUM") as scpool,
            tc.tile_pool(name="acc", bufs=2, space="PSUM") as accpool,
        ):
            # ---- inputs / constants ----
            # xt split into column halves, qkvw into 4 m-column groups + v, so
            # the first matmuls only wait on the slices they need.
            xT_ap = xT_d[:].rearrange("(t p) n -> p t n", p=128)
            xts = []
            for n2 in range(2):
                xth = cpool.tile([128, CT, 512], bf16, tag=f"xt{n2}", name=f"xt{n2}")
                nc.sync.dma_start(xth[:], xT_ap[:, :, n2 * 512 : (n2 + 1) * 512])
                xts.append(xth)
            qkvw_ap = qkvw_d[:].rearrange("(t p) n -> p t n", p=128)
            qkws = []
            for g in range(4):
                w = cpool.tile([128, CT, 256], bf16, tag=f"qkw{g}", name=f"qkw{g}")
                nc.sync.dma_start(w[:], qkvw_ap[:, :, g * 256 : (g + 1) * 256])
                qkws.append(w)
            qkbr = cpool.tile([1, 2 * C], bf16)
            nc.sync.dma_start(qkbr[:], qkbr_d[:])
            bvc = cpool.tile([128, 4], f32)
            nc.sync.dma_start(bvc[:], bvc_d[:])
            pjb = cpool.tile([128, 4], f32)
            nc.sync.dma_start(pjb[:], pjb_d[:])
            ones = cpool.tile([1, 512], bf16)
            nc.gpsimd.memset(ones[:], 1.0)
            vw = cpool.tile([128, CT, C], bf16, tag="vw", name="vw")
            nc.sync.dma_start(vw[:], qkvw_ap[:, :, 2 * C : 3 * C])

            keep_ap = keep_d[:].rearrange("h (kp p) q -> p h kp q", p=128)
            keeps = {}

            def prefetch_keep(h):
                for q4 in range(4):
                    keep2 = kpool.tile(
                        [128, 2, N], bf16, tag="keep", name=f"keep{h}_{q4}"
                    )
                    nc.sync.dma_start(keep2[:], keep_ap[:, h, q4 * 2 : q4 * 2 + 2, :])
                    keeps[(h, q4)] = keep2

            prefetch_keep(1)
            prefetch_keep(0)

            # ---- QKV projections (bias via K=1 rank-1 accumulate) ----
            # one SBUF tile per m so scores of early heads never falsely
            # serialize against later m-tile evacuations (tile-granular deps)
            qkTs = [None] * 8

            qk_ps = {}

            def emit_qk_half(m, half):
                if half == 0:
                    qk_ps[m] = accpool.tile([128, N], f32, tag="acc", name=f"qkps{m}")
                ps = qk_ps[m]
                for t in (0, 1) if half == 0 else (2, 3):
                    lhsT = qkws[m // 2][:, t, (m % 2) * 128 : (m % 2) * 128 + 128]
                    for n2 in range(2):
                        nc.tensor.matmul(
                            ps[:, n2 * 512 : (n2 + 1) * 512],
                            lhsT,
                            xts[n2][:, t, :],
                            start=(t == 0),
                            stop=False,
                        )
                if half == 1:
                    for n2 in range(2):
                        nc.tensor.matmul(
                            ps[:, n2 * 512 : (n2 + 1) * 512],
                            qkbr[0:1, m * 128 : (m + 1) * 128],
                            ones[0:1, :],
                            start=False,
                            stop=True,
                        )
                    qkTs[m] = qkpool.tile(
                        [128, N], bf16, tag=f"qk{m}", name=f"qkT{m}"
                    )
                    nc.scalar.activation(qkTs[m][:], ps[:], ID, bias=0.0, scale=1.0)

            def emit_qk_tile(m):
                emit_qk_half(m, 0)
                emit_qk_half(m, 1)

            emit_qk_tile(0)
            emit_qk_tile(4)

            # v token-major with ones column: v[tok, head, 66]
            # (matmuls emitted inside head-0's loop as PE filler work)
            vsbs = [None] * NT

            def emit_v_tile(mt):
                psv = accpool.tile([128, N], f32, tag="acc", name=f"vps{mt}")
                for t in range(CT):
                    nc.tensor.matmul(
                        psv[:, 0:512],
                        xts[mt // 4][:, t, (mt % 4) * 128 : (mt % 4) * 128 + 128],
                        vw[:, t, :],
                        start=(t == 0),
                        stop=(t == CT - 1),
                    )
                vsb = vpool.tile([128, H, 66], bf16, tag=f"v{mt}", name=f"vsb{mt}")
                vsbs[mt] = vsb
                nc.gpsimd.memset(vsb[:, :, 64:65], 1.0)
                nc.gpsimd.memset(vsb[:, :, 65:66], 0.0)
                if mt % 2 == 0:
                    nc.scalar.activation(
                        vsb[:, :, 0:64],
                        psv[:, 0:512].rearrange("p (h d) -> p h d", d=64),
                        ID,
                        bias=0.0,
                        scale=1.0,
                    )
                else:
                    nc.vector.tensor_copy(
                        vsb[:, :, 0:64],
                        psv[:, 0:512].rearrange("p (h d) -> p h d", d=64),
                    )

            # ---- attention: software-pipelined heads, kt granularity ----
            aTs = [
                apool.tile([128, N], bf16, tag=f"aT{t}", name=f"aT{t}")
                for t in range(CT)
            ]
            Ss = [
                spool.tile([128, N], f32, tag=f"S{t}", name=f"S{t}") for t in range(CT)
            ]
            pts = {}  # (h, kt) -> [128, N] bf16 masked-p tile
            psos = [None] * H

            def emit_score_kt(h, kt):
                pbase = (h % 2) * 64
                qT_h = qkTs[h // 2][pbase : pbase + 64, :]
                kT_h = qkTs[4 + h // 2][pbase : pbase + 64, :]
                ps_s = scpool.tile([128, N], f32, tag="sc", name=f"sc{h}_{kt}")
                for n2 in range(2):
                    nc.tensor.matmul(
                        ps_s[:, n2 * 512 : (n2 + 1) * 512],
                        kT_h[:, kt * 128 : (kt + 1) * 128],
                        qT_h[:, n2 * 512 : (n2 + 1) * 512],
                        start=True,
                        stop=True,
                    )
                pt = ppool.tile([128, N], bf16, tag="pT", name=f"pT{h}_{kt}")
                pts[(h, kt)] = pt
                nc.scalar.activation(
                    pt[:],
                    ps_s[:],
                    mybir.ActivationFunctionType.Exp,
                    bias=0.0,
                    scale=float(SCALE),
                )
                eng = nc.gpsimd if kt in GP_KT else nc.vector
                eng.tensor_tensor(pt[:], pt[:], keeps[(h, kt // 2)][:, kt % 2, :], MULT)

            def emit_pv_kt(h, kt):
                if kt == 0:
                    psos[h] = accpool.tile([128, N], f32, tag="acc", name=f"pso{h}")
                ps_o = psos[h]
                for n2 in range(2):
                    nc.tensor.matmul(
                        ps_o[0:66, n2 * 512 : (n2 + 1) * 512],
                        vsbs[kt][:, h, :],
                        pts[(h, kt)][:, n2 * 512 : (n2 + 1) * 512],
                        start=(kt == 0),
                        stop=(kt == NT - 1 and n2 == 1),
                    )
                if kt == NT - 1:
                    del pts[(h, kt)]

            def emit_head_tail(h, norm_pair):
                """rowsum -> partition 0, broadcast into S block, evac out^T."""
                ps_o = psos[h]
                pbase = (h % 2) * 64
                rs = rspool.tile([1, N], f32, tag="rs")
                nc.vector.tensor_copy(rs[0:1, :], ps_o[64:65, :])
                if pbase == 0:
                    nc.gpsimd.partition_broadcast(Ss[h // 2][0:64, :], rs[0:1, :])
                else:
                    rb = rspool.tile([64, N], f32, tag="rb")
                    nc.gpsimd.partition_broadcast(rb[:], rs[0:1, :])
                    nc.sync.dma_start(Ss[h // 2][64:128, :], rb[:])
                nc.vector.tensor_copy(aTs[h // 2][pbase : pbase + 64, :], ps_o[0:64, :])
                psos[h] = None
                if norm_pair:
                    # heads 2t and 2t+1 have filled c-tile t: normalize it now
                    t = h // 2
                    nc.vector.reciprocal_approx_fast(Ss[t][:], Ss[t][:])
                    nc.vector.tensor_tensor(aTs[t][:], aTs[t][:], Ss[t][:], MULT)
                    nc.vector.tensor_scalar_add(aTs[t][:], aTs[t][:], bvc[:, t : t + 1])

            # head order: pair-swapped so the final head is even (cheaper
            # tail chain) and the last head's PV self-interleaves at lag 3
            ORDER = [1, 0, 3, 2, 5, 4, 7, 6]
            QK_FOLD = {0: 1, 1: 5, 2: 2, 3: 6, 4: 3, 5: 7}
            for i, h in enumerate(ORDER):
                if i + 2 < H:
                    prefetch_keep(ORDER[i + 2])
                last = i == H - 1
                for kt in range(NT):
                    emit_score_kt(h, kt)
                    if i == 0:
                        emit_v_tile(kt)
                    else:
                        emit_pv_kt(ORDER[i - 1], kt)
                    if last and kt >= 2:
                        emit_pv_kt(h, kt - 2)
                    if kt == 3 and i in QK_FOLD:
                        emit_qk_half(QK_FOLD[i], 0)
                    if kt == 6 and i in QK_FOLD:
                        emit_qk_half(QK_FOLD[i], 1)
                if i > 0:
                    emit_head_tail(ORDER[i - 1], i % 2 == 0)
            for kt in range(NT - 2, NT):
                emit_pv_kt(ORDER[-1], kt)
            emit_head_tail(ORDER[-1], True)

            # ---- output projection ----
            projw = cpool.tile([128, CT, C], bf16, tag="projw", name="projw")
            nc.sync.dma_start(
                projw[:], projw_d[:].rearrange("(t p) n -> p t n", p=128)
            )
            # t-outer accumulation into 4 parallel PSUM tiles so the t<3
            # matmuls overlap the final head's tail/normalization.
            ps_ys = []
            for mo in range(CT):
                pool = scpool if mo < 2 else accpool
                tag = "sc" if mo < 2 else "acc"
                ps_ys.append(pool.tile([128, N], f32, tag=tag, name=f"yps{mo}"))
            for t in range(CT):
                for mo in range(CT):
                    lhsT = projw[:, t, mo * 128 : (mo + 1) * 128]
                    for n2 in range(2):
                        nc.tensor.matmul(
                            ps_ys[mo][:, n2 * 512 : (n2 + 1) * 512],
                            lhsT,
                            aTs[t][:, n2 * 512 : (n2 + 1) * 512],
                            start=(t == 0),
                            stop=(t == CT - 1),
                        )
            for mo in range(CT):
                yT = ypool.tile([128, N], f32, tag="yT", name=f"yT{mo}", bufs=4)
                if mo % 2 == 0:
                    nc.scalar.activation(
                        yT[:], ps_ys[mo][:], ID, bias=pjb[:, mo : mo + 1], scale=1.0
                    )
                else:
                    nc.vector.tensor_scalar_add(
                        yT[:], ps_ys[mo][:], pjb[:, mo : mo + 1]
                    )
                nc.sync.dma_start(
                    out_d[:].rearrange("(t p) n -> p t n", p=128)[:, mo, :], yT[:]
                )

    nc.compile()
    return nc


def _get_nc():
    if "nc" not in _cache:
        _cache["nc"] = _build_nc()
    return _cache["nc"]


def _get_keepT():
    """keepT[b, h, k, q] = 1 - mask[b, h, q, k], bf16, one array per batch."""
    if "keepT" not in _cache:
        import jax

        mask = np.asarray(jax.random.bernoulli(jax.random.key(42), 0.5, (B, H, N, N)))
        keep = (~mask).astype(_bf16())
        _cache["keepT"] = np.ascontiguousarray(keep.transpose(0, 1, 3, 2))
    return _cache["keepT"]


def kernel(x, qkv_w, qkv_b, proj_w, proj_b):
    from concourse.bass_utils import run_bass_kernel_spmd

    bf16 = _bf16()
    x = np.asarray(x, dtype=np.float32)
    qkv_w = np.asarray(qkv_w, dtype=np.float32)
    qkv_b = np.asarray(qkv_b, dtype=np.float32)
    proj_w = np.asarray(proj_w, dtype=np.float32)
    proj_b = np.asarray(proj_b, dtype=np.float32)

    nc = _get_nc()
    keepT = _get_keepT()

    qkvw_t = np.ascontiguousarray(qkv_w.T).astype(bf16)  # [C, 3C]
    projw_t = np.ascontiguousarray(proj_w.T).astype(bf16)  # [C, C]
    qkbr = qkv_b[:1024].astype(bf16).reshape(1, 2 * C)
    bvc = np.ascontiguousarray(qkv_b[1024:].reshape(4, 128).T)  # [128, 4]
    pjb = np.ascontiguousarray(proj_b.reshape(4, 128).T)  # [128, 4]

    in_maps = []
    for b in range(B):
        in_maps.append(
            {
                "xT": np.ascontiguousarray(x[b].T).astype(bf16),
                "qkvw": qkvw_t,
                "projw": projw_t,
                "keepT": keepT[b],
                "qkbr": qkbr,
                "bvc": bvc,
                "pjb": pjb,
            }
        )

    global _last_in_maps
    _last_in_maps = in_maps
    res = run_bass_kernel_spmd(nc, in_maps, core_ids=list(range(8)))
    out = np.stack([res.results[b]["out"].T for b in range(B)])  # [B, N, C]
    return out.astype(np.float32)


_last_in_maps = None
